# revision 2
# baseline (speedup 1.0000x reference)
"""PointNet++ binary segmentation kernel for 8 Trainium2 NeuronCores.

Strategy: pure data parallelism — the batch dim (16) is sharded 2-per-core
across the 8 cores.  FPS, kNN, grouping, and the pointwise MLPs are
batch-independent; the only cross-core coupling is BatchNorm (training
mode, batch statistics over the FULL batch), which is handled with
cross-core mean reductions (lax.pmean) inside the per-core program.

The whole forward pass runs on-device via jax.pmap over the 8 axon-tunneled
NeuronCores; parameters are replicated.
"""

import numpy as np

EPS_BN = 1e-5
K_NN = 16
NCORES = 8

_COMPILED = {}


def _build():
    import jax
    import jax.numpy as jnp
    from jax import lax

    def _bn_global(x, g, be, axes, axis_name):
        # Global batch statistics across all 16 batch elements (8 cores x 2).
        # mean/var over local axes, then cross-core mean.  Shards are equal
        # sized, so pmean of shard-means == global mean.
        m1 = lax.pmean(jnp.mean(x, axes, keepdims=True), axis_name)
        m2 = lax.pmean(jnp.mean(x * x, axes, keepdims=True), axis_name)
        var = m2 - m1 * m1
        return (x - m1) * lax.rsqrt(var + EPS_BN) * g + be

    def _mlp(x, layers, axes, axis_name):
        for p in layers:
            x = jnp.einsum("...i,oi->...o", x, p["W"]) + p["b"]
            x = jax.nn.relu(_bn_global(x, p["g"], p["be"], axes, axis_name))
        return x

    def _gather(points, idx):
        return jax.vmap(lambda p, i: p[i])(points, idx)

    def _sqdist(src, dst):
        return jnp.sum((src[:, :, None, :] - dst[:, None, :, :]) ** 2, -1)

    def _fps(xyz, npoint):
        B, N, _ = xyz.shape
        dist0 = jnp.full((B, N), 1e10, xyz.dtype)
        far0 = jnp.zeros((B,), jnp.int32)

        def step(carry, _):
            distance, farthest = carry
            centroid = jnp.take_along_axis(xyz, farthest[:, None, None], axis=1)
            d = jnp.sum((xyz - centroid) ** 2, -1)
            distance = jnp.minimum(distance, d)
            return (distance, jnp.argmax(distance, -1).astype(jnp.int32)), farthest

        _, idxs = lax.scan(step, (dist0, far0), None, length=npoint)
        return idxs.T

    def _sa(xyz, points, npoint, layers, axis_name):
        fps_idx = _fps(xyz, npoint)
        new_xyz = _gather(xyz, fps_idx)
        d = _sqdist(new_xyz, xyz)
        _, idx = lax.top_k(-d, K_NN)
        grouped_xyz = _gather(xyz, idx) - new_xyz[:, :, None, :]
        if points is not None:
            new_points = jnp.concatenate([grouped_xyz, _gather(points, idx)], -1)
        else:
            new_points = grouped_xyz
        new_points = _mlp(new_points, layers, (0, 1, 2), axis_name)
        return new_xyz, jnp.max(new_points, axis=2)

    def _fp(xyz1, xyz2, points1, points2, layers, axis_name):
        d = _sqdist(xyz1, xyz2)
        negd, idx = lax.top_k(-d, 3)
        w = 1.0 / (-negd + 1e-8)
        w = w / jnp.sum(w, -1, keepdims=True)
        interp = jnp.sum(_gather(points2, idx) * w[..., None], axis=2)
        new_points = interp if points1 is None else jnp.concatenate([points1, interp], -1)
        return _mlp(new_points, layers, (0, 1), axis_name)

    def forward(x, params):
        ax = "b"
        l0_xyz = x.astype(jnp.float32)
        l1_xyz, l1_points = _sa(l0_xyz, None, 512, params["sa1"], ax)
        l2_xyz, l2_points = _sa(l1_xyz, l1_points, 128, params["sa2"], ax)
        l3_xyz, l3_points = _sa(l2_xyz, l2_points, 32, params["sa3"], ax)
        l2_points = _fp(l2_xyz, l3_xyz, l2_points, l3_points, params["fp3"], ax)
        l1_points = _fp(l1_xyz, l2_xyz, l1_points, l2_points, params["fp2"], ax)
        l0_points = _fp(l0_xyz, l1_xyz, None, l1_points, params["fp1"], ax)
        p1 = params["cls1"]
        h = jnp.einsum("bnc,oc->bno", l0_points, p1["W"]) + p1["b"]
        h = jax.nn.relu(_bn_global(h, p1["g"], p1["be"], (0, 1), ax))
        p2 = params["cls2"]
        logits = jnp.einsum("bnc,oc->bno", h, p2["W"]) + p2["b"]
        return logits

    return forward


def _get_compiled(n_dev):
    key = n_dev
    if key in _COMPILED:
        return _COMPILED[key]
    import jax

    forward = _build()
    fn = jax.pmap(forward, axis_name="b", in_axes=(0, None))
    _COMPILED[key] = fn
    return fn


def _kernel_pmap(x, params_np):
    import jax

    B, N, _ = x.shape
    bpc = B // NCORES
    xs = x.reshape(NCORES, bpc, N, 3)
    fn = _get_compiled(NCORES)
    out = fn(xs, params_np)  # [8, bpc, N, 2]
    return np.asarray(out, dtype=np.float32).reshape(B, N, 2)


def _kernel_single(x, params_np):
    # Fallback: whole batch on one device, exact reference semantics
    # (global BN is trivially satisfied on a single device).
    import jax
    import jax.numpy as jnp
    from jax import lax

    if "single" not in _COMPILED:
        forward = _build()

        def fwd1(x, params):
            # run the pmap-style forward with a dummy size-1 axis by
            # re-binding pmean to identity via a fake axis of size 1
            return jax.pmap(forward, axis_name="b", in_axes=(0, None))(
                x[None], params
            )[0]

        _COMPILED["single"] = fwd1
    return np.asarray(_COMPILED["single"](x, params_np), dtype=np.float32)


def kernel(x, params):
    import jax

    x = np.asarray(x, dtype=np.float32)
    B, N, _ = x.shape
    assert B % NCORES == 0, (B, NCORES)

    # host-side param tree -> plain numpy (pmap replicates via in_axes=None)
    params_np = jax.tree_util.tree_map(lambda a: np.asarray(a, dtype=np.float32), params)

    try:
        n_dev = len(jax.devices())
    except Exception:
        n_dev = 1
    if n_dev >= NCORES:
        try:
            return _kernel_pmap(x, params_np)
        except Exception as e:  # pragma: no cover - defensive fallback
            import traceback

            traceback.print_exc()
            print(f"kernel: pmap path failed ({e!r}); falling back to single-device")
    return _kernel_single(x, params_np)


if __name__ == "__main__":
    # tiny smoke run against the local reference if present
    import sys

    sys.path.insert(0, "/root/problem")
    import reference

    inputs = reference.setup_inputs()
    got = kernel(**{k: np.asarray(v) if not isinstance(v, dict) else v for k, v in inputs.items()})
    exp = np.asarray(reference.reference(**inputs))
    rel = np.linalg.norm(got - exp) / np.linalg.norm(exp)
    print("Relative error:", rel)
